# revision 1
# baseline (speedup 1.0000x reference)
"""Trainium2 Bass kernel for nn_DecGreenNet_product_CP3.

Reference computation:
    lhs  = tanh(input @ Wx1 + bx1) @ Wx2 + bx2          # [B, 512]
    s_i  = sum_n sin(pi*eq*qx_n) * mlp_i(qx_n)           # [8,16] per branch
    rhs  = einsum('bx,dx,fx->bdf', s_a, s_c, s_e)        # [512]
    out  = lhs @ rhs                                     # [B]

Algebraic restructuring used here (validated to ~2e-6 rel err):
    out[b] = tanh(input[b] @ Wx1 + bx1) @ (Wx2 @ rhs) + bx2 @ rhs
    s      = W2^T @ (h1tanh^T @ y) + (sum y) * b2   per quad branch
collapsing the dominant [B,512]x[512,512] GEMM into a matvec.

Sharding: batch B split 8 ways (8192 rows/core); quad nodes split 8 ways
(1024 nodes/core) with a tiny [128,4] AllReduce of the per-core partial
s-vectors (the branch reduction is linear, so partials sum exactly).
"""

import numpy as np

import concourse.bacc as bacc
import concourse.bass as bass
import concourse.mybir as mybir
import concourse.tile as tile
from concourse.bass_utils import run_bass_kernel_spmd

F32 = mybir.dt.float32
F16 = mybir.dt.float16
AF = mybir.ActivationFunctionType
ALU = mybir.AluOpType

NCORES = 8
B, DIN, H = 65536, 3, 512
N, HQ = 8192, 128
S0, RX = 8, 16
BL = B // NCORES          # 8192 batch rows per core
NL = N // NCORES          # 1024 quad nodes per core
NT = NL // 128            # 8 node tiles per branch
CH = 512                  # batch chunk (columns per matmul)
NCH = BL // CH            # 16 chunks
HTILES = H // 128         # 4 h tiles

# scheduling knobs
EMIT_BEFORE = 16          # L1 chunks emitted before the post-collective block
HID_BUFS = 32             # keep all hidden tiles resident

# fp16 scaling: w values are ~1e10-1e11; scale into fp16 range (exact pow2)
RC_SCALE = 2.0 ** -36     # applied to rhs_vec before the fp16 w-matmuls
W_SCALE = 1.0             # applied on psum->sbuf copy of w (total 2^-36)
OUT_SCALE = 2.0 ** 36     # undo in the final output pass

# minimax odd polynomial for sin(t), t in [0, pi]: sin(t)=t*P(t^2), err<2e-5
SIN_C = (0.999984590176674, -0.16663258473611252, 8.312385898666645e-03,
         -1.9316230946716391e-04, 2.1732361127812407e-06)

_CACHED_NC = None

import os
_STAGE = os.environ.get("K_STAGE", "full")  # quad | cc | eins | mainonly | full


def _build():
    nc = bacc.Bacc("TRN2", target_bir_lowering=False, debug=False,
                   num_devices=NCORES)

    xT = nc.dram_tensor("xT", [DIN + 1, BL], F16, kind="ExternalInput").ap()
    wx1a = nc.dram_tensor("wx1a", [DIN + 1, H], F16, kind="ExternalInput").ap()
    wx2t = nc.dram_tensor("wx2tb", [64, 4096], F16, kind="ExternalInput").ap()
    bx2r = nc.dram_tensor("bx2rb", [64, 128], F16, kind="ExternalInput").ap()
    qxa = nc.dram_tensor("qxa", [6, NL], F16, kind="ExternalInput").ap()
    qxc = nc.dram_tensor("qxc", [128, 3 * NT], F32, kind="ExternalInput").ap()
    wqa = nc.dram_tensor("wqa", [6, HQ], F16, kind="ExternalInput").ap()
    wq2 = nc.dram_tensor("wq2", [HQ, 3 * HQ], F32, kind="ExternalInput").ap()
    bq2r = nc.dram_tensor("bq2r", [3, HQ], F32, kind="ExternalInput").ap()
    eqb = nc.dram_tensor("eqb", [128, 1], F32, kind="ExternalInput").ap()
    out_d = nc.dram_tensor("out", [BL], F32, kind="ExternalOutput").ap()

    global _APS
    _APS = (xT, wx1a, wx2t, bx2r, qxa, qxc, wqa, wq2, bq2r, eqb, out_d)
    with tile.TileContext(nc) as tc:
        _body(nc, tc)
    nc.compile()
    return nc


def _body(nc, tc):
        xT, wx1a, wx2t, bx2r, qxa, qxc, wqa, wq2, bq2r, eqb, out_d = _APS
        with (
            tc.tile_pool(name="const", bufs=1) as constp,
            tc.tile_pool(name="qsb", bufs=1) as qsb,
            tc.tile_pool(name="h1p", bufs=4) as h1p,
            tc.tile_pool(name="dram", bufs=2, space="DRAM") as dram,
            tc.tile_pool(name="tinyp", bufs=1, space="PSUM") as tinyp,
            tc.tile_pool(name="mainsb", bufs=1) as mainsb,
            tc.tile_pool(name="orowp", bufs=3) as orowp,
            tc.tile_pool(name="esb", bufs=2) as esb,
            tc.tile_pool(name="hidp", bufs=HID_BUFS) as hidp,
            tc.tile_pool(name="prep", bufs=2, space="PSUM") as prep,
            tc.tile_pool(name="outp", bufs=2, space="PSUM") as outpp,
        ):
            ones128 = constp.tile([128, 1], F32)
            nc.vector.memset(ones128, 1.0)

            # ---------------- quad phase DMAs ----------------
            # per-branch tiles so every matmul operand starts at partition 0
            # y-polynomial inputs first (critical path to the collective)
            qxc_sb = qsb.tile([128, 3 * NT], F32, tag="qxc")
            nc.sync.dma_start(out=qxc_sb, in_=qxc)
            eqb_sb = qsb.tile([128, 1], F32, tag="eqb")
            nc.sync.dma_start(out=eqb_sb, in_=eqb)
            qxa_sb, wqa_sb, bq2r_sb = [], [], []
            qeng = [nc.gpsimd, nc.sync, nc.gpsimd]
            for br in range(3):
                e = qeng[br]
                t = qsb.tile([2, NL], F16, tag=f"qxa{br}")
                e.dma_start(out=t, in_=qxa[2 * br:2 * br + 2, :])
                qxa_sb.append(t)
                t = qsb.tile([2, HQ], F16, tag=f"wqa{br}")
                e.dma_start(out=t, in_=wqa[2 * br:2 * br + 2, :])
                wqa_sb.append(t)
                t = qsb.tile([1, HQ], F32, tag=f"bq2r{br}")
                e.dma_start(out=t, in_=bq2r[br:br + 1, :])
                bq2r_sb.append(t)
            wq2_sb = qsb.tile([HQ, 3 * HQ], F32, tag="wq2")
            nc.sync.dma_start(out=wq2_sb, in_=wq2)

            qcut = int(os.environ.get("K_QCUT", "99"))

            def qdump(ap2d):
                p, c = ap2d.shape[0], ap2d.shape[1]
                nc.sync.dma_start(
                    out=out_d[0:p * c].rearrange("(p c) -> p c", c=c),
                    in_=ap2d)

            if qcut <= 1:
                qdump(qxc_sb[:, 0:3])
                return

            # y = sin(pi*eq*qx) via odd minimax polynomial on the DVE
            # (keeps ScalarE on a single act-table set: Tanh only)
            eqpi = qsb.tile([128, 1], F32, tag="eqpi")
            nc.vector.tensor_scalar_mul(eqpi, eqb_sb, float(np.pi))
            tq = qsb.tile([128, 3 * NT], F32, tag="tq")
            nc.vector.tensor_scalar_mul(tq, qxc_sb, eqpi[:, 0:1])
            t2 = qsb.tile([128, 3 * NT], F32, tag="t2")
            nc.vector.tensor_tensor(out=t2, in0=tq, in1=tq, op=ALU.mult)
            pp = qsb.tile([128, 3 * NT], F32, tag="pp")
            c1, c3, c5, c7, c9 = [float(v) for v in SIN_C]
            nc.vector.tensor_scalar(out=pp, in0=t2, scalar1=c9, scalar2=c7,
                                    op0=ALU.mult, op1=ALU.add)
            for cof in (c5, c3, c1):
                nc.vector.tensor_tensor(out=pp, in0=pp, in1=t2, op=ALU.mult)
                nc.vector.tensor_scalar_add(pp, pp, cof)
            y_sb = qsb.tile([128, 3 * NT], F16, tag="ysb")
            nc.vector.tensor_tensor(out=y_sb, in0=pp, in1=tq, op=ALU.mult)
            if qcut <= 2:
                qdump(y_sb[:, 0:3])
                return

            # ---------------- quad branches ----------------
            # qsmall columns: 0-2 = z per branch, 3-5 = sy per branch (row 0),
            # 6-8 = s per branch
            qsmall = tinyp.tile([128, 12], F32, tag="tiny")
            nc.vector.memset(qsmall[:, 3:6], 0.0)
            for br in range(3):
                h1s = []
                for half in range(2):
                    qpre = prep.tile([128, 512], F32, tag="pre")
                    for i2 in range(4):
                        i = half * 4 + i2
                        nc.tensor.matmul(
                            qpre[:, i2 * 128:(i2 + 1) * 128],
                            lhsT=qxa_sb[br][:, i * 128:(i + 1) * 128],
                            rhs=wqa_sb[br],
                            start=True, stop=True)
                    h1 = h1p.tile([128, 512], F16, tag="h1")
                    nc.scalar.activation(out=h1, in_=qpre, func=AF.Tanh)
                    h1s.append(h1)
                # z[h] = sum_n h1[n,h]*y[n], accumulated over 8 node tiles
                for i in range(NT):
                    nc.tensor.matmul(
                        qsmall[:, br:br + 1],
                        lhsT=h1s[i // 4][:, (i % 4) * 128:(i % 4 + 1) * 128],
                        rhs=y_sb[:, br * NT + i:br * NT + i + 1],
                        start=(i == 0), stop=(i == NT - 1))
                if qcut <= 5:
                    continue
                # sy = sum_n y[n]  -> row 0 of column 3+br
                ysum = qsb.tile([128, 1], F32, tag="ysum")
                nc.vector.tensor_reduce(
                    out=ysum, in_=y_sb[:, br * NT:(br + 1) * NT],
                    axis=mybir.AxisListType.X, op=ALU.add)
                nc.tensor.matmul(
                    qsmall[0:1, 3 + br:4 + br], lhsT=ysum[:, 0:1],
                    rhs=ones128[:, 0:1], start=True, stop=True)

            if qcut <= 3:
                qdump(h1s[0][:, 0:3])
                return

            z_sb = qsb.tile([128, 6], F32, tag="zsb")
            if qcut <= 5:
                nc.vector.tensor_copy(out=z_sb[:, 0:3], in_=qsmall[:, 0:3])
                qdump(z_sb[:, 0:3])
                return
            nc.vector.tensor_copy(out=z_sb, in_=qsmall[:, 0:6])
            if qcut <= 6:
                qdump(z_sb[:, 0:6])
                return
            # s = W2^T z + sy * b2 per branch -> columns 6..8
            for br in range(3):
                nc.tensor.matmul(
                    qsmall[:, 6 + br:7 + br],
                    lhsT=wq2_sb[:, br * HQ:(br + 1) * HQ],
                    rhs=z_sb[:, br:br + 1], start=True, stop=False)
                nc.tensor.matmul(
                    qsmall[:, 6 + br:7 + br],
                    lhsT=bq2r_sb[br],
                    rhs=z_sb[0:1, 3 + br:4 + br], start=False, stop=True)
            s_sb = qsb.tile([128, 3], F32, tag="ssb")
            nc.vector.tensor_copy(out=s_sb, in_=qsmall[:, 6:9])

            if _STAGE == "quad":
                nc.sync.dma_start(out=out_d[0:384],
                                  in_=s_sb.rearrange("p c -> (p c)"))
                return

            # ---------------- AllReduce of partial s ----------------
            # bounce buffers hold s already transposed to [16 x, (br, b)] so
            # the post-barrier read is a dense [16, 24] block
            cc_in = dram.tile([16, 24], F32, tag="ccin")
            cc_out = dram.tile([16, 24], F32, tag="ccout")
            nc.gpsimd.dma_start(out=cc_in.rearrange("x (c b) -> b x c", b=8),
                                in_=s_sb)
            nc.gpsimd.collective_compute(
                "AllReduce", ALU.add,
                replica_groups=[list(range(NCORES))],
                ins=[cc_in[:].opt()], outs=[cc_out[:].opt()])
            if _STAGE == "cc":
                sg_sb = qsb.tile([16, 24], F32, tag="sgsb")
                nc.gpsimd.dma_start(out=sg_sb, in_=cc_out)
                qdump(sg_sb)
                return

            # ---------------- main phase DMAs ----------------
            xT_sb = mainsb.tile([DIN + 1, BL], F16, tag="xT")
            nc.sync.dma_start(out=xT_sb, in_=xT)
            wx1a_sb = mainsb.tile([DIN + 1, H], F16, tag="wx1a")
            nc.sync.dma_start(out=wx1a_sb, in_=wx1a)
            wx2t_sb = mainsb.tile([64, 4096], F16, tag="wx2t")
            nc.sync.dma_start(out=wx2t_sb, in_=wx2t)
            bx2r_sb = mainsb.tile([64, 128], F16, tag="bx2r")
            nc.sync.dma_start(out=bx2r_sb, in_=bx2r)

            # ---------------- main L1 chunks (emitter) ----------------
            hid_tiles = {}

            def emit_l1(c):
                tiles = []
                for half in range(2):
                    pre = prep.tile([128, 1024], F32, tag="pre")
                    for k in range(2):
                        ht = half * 2 + k
                        nc.tensor.matmul(
                            pre[:, k * 512:(k + 1) * 512],
                            lhsT=wx1a_sb[:, ht * 128:(ht + 1) * 128],
                            rhs=xT_sb[:, c * CH:(c + 1) * CH],
                            start=True, stop=True)
                    hid = hidp.tile([128, 1024], F16, tag="hid")
                    nc.scalar.activation(out=hid, in_=pre, func=AF.Tanh)
                    tiles.append(hid)
                hid_tiles[c] = tiles

            for c in range(EMIT_BEFORE):
                emit_l1(c)

            # ---------------- post-collective small compute ----------------
            # s columns [128]=(b*16+x) -> sT[16x, (br,8b)] straight from the
            # collective's DRAM output (single strided DMA)
            sT_sb = esb.tile([16, 24], F32, tag="sT")
            nc.sync.dma_start(out=sT_sb, in_=cc_out)
            # E[x, d*8+f] = s_c[d,x] * s_e[f,x]
            sc_ap = sT_sb[:, 8:16]
            se_ap = sT_sb[:, 16:24]
            in0 = bass.AP(tensor=sc_ap.tensor, offset=sc_ap.offset,
                          ap=[sc_ap.ap[0], sc_ap.ap[1], [0, 8]])
            in1 = bass.AP(tensor=se_ap.tensor, offset=se_ap.offset,
                          ap=[se_ap.ap[0], [0, 8], se_ap.ap[1]])
            E_sb = esb.tile([16, 64], F32, tag="E")
            nc.vector.tensor_tensor(
                out=E_sb.rearrange("p (d f) -> p d f", f=8),
                in0=in0, in1=in1, op=ALU.mult)
            # rhs_vec[b, d*8+f] = sum_x s_a[b? -> see below] ;
            # out[b,df] = sum_x sT_a[x,b] * E[x,df]
            rhsp = tinyp.tile([64, 8], F32, tag="tiny")
            nc.tensor.matmul(rhsp, lhsT=E_sb, rhs=sT_sb[:, 0:8],
                             start=True, stop=True)
            r16 = esb.tile([64, 8], F16, tag="r16")
            nc.vector.tensor_scalar_mul(r16, rhsp, float(RC_SCALE))
            if _STAGE == "eins2":
                qdump(r16)
                return
            # w = Wx2 @ rhs_vec as [128, 4] (h = it*128+p), contracted over
            # b-blocks of 64 straight from the [64 df, 8 b] einsum layout
            wps = tinyp.tile([128, 4], F32, tag="tiny")
            for it in range(4):
                for b in range(8):
                    nc.tensor.matmul(
                        wps[:, it:it + 1],
                        lhsT=wx2t_sb[:, b * 512 + it * 128:b * 512 + (it + 1) * 128],
                        rhs=r16[:, b:b + 1],
                        start=(b == 0), stop=(b == 7))
            w_sb = esb.tile([128, 4], F16, tag="wsb")
            nc.vector.tensor_scalar_mul(w_sb, wps, float(W_SCALE))
            # c (scalar, scaled by RC_SCALE) replicated over 16 partitions
            c16p = tinyp.tile([16, 1], F32, tag="tiny")
            for b in range(8):
                nc.tensor.matmul(
                    c16p, lhsT=bx2r_sb[:, b * 16:(b + 1) * 16],
                    rhs=r16[:, b:b + 1],
                    start=(b == 0), stop=(b == 7))
            c16_sb = esb.tile([16, 1], F32, tag="c16")
            nc.vector.tensor_copy(out=c16_sb, in_=c16p)

            # ---------------- rest of L1 + dots ----------------
            def emit_dot(c):
                op = outpp.tile([1, 512], F32, tag="outp")
                for ht in range(HTILES):
                    nc.tensor.matmul(
                        op,
                        lhsT=w_sb[:, ht:ht + 1],
                        rhs=hid_tiles[c][ht // 2][:, (ht % 2) * 512:(ht % 2 + 1) * 512],
                        start=(ht == 0), stop=(ht == HTILES - 1))
                orow = orowp.tile([1, 512], F32, tag="outrow")
                nc.vector.tensor_scalar(
                    out=orow, in0=op, scalar1=c16_sb[0:1, 0:1],
                    scalar2=float(OUT_SCALE), op0=ALU.add, op1=ALU.mult)
                nc.sync.dma_start(
                    out=out_d[c * CH:(c + 1) * CH].rearrange("(o b) -> o b", o=1),
                    in_=orow)

            for c in range(EMIT_BEFORE, NCH):
                emit_l1(c)
            for c in range(NCH):
                emit_dot(c)


def _get_nc():
    global _CACHED_NC
    if _CACHED_NC is None:
        _CACHED_NC = _build()
    return _CACHED_NC


def _prep_in_maps(inputs):
    f = lambda k: np.ascontiguousarray(np.asarray(inputs[k], np.float32))
    inputx = f("input")
    eq = float(np.asarray(inputs["eq_param"]).reshape(-1)[0])
    Wx1, bx1 = f("Wx1"), f("bx1")
    Wx2, bx2 = f("Wx2"), f("bx2")

    wx1a = np.concatenate([Wx1, bx1[None, :]], axis=0).astype(np.float16)
    # wx2tb[df, b*512+it*128+i] = Wx2T[b*64+df, it*128+i]
    wx2tb = np.ascontiguousarray(
        Wx2.T.reshape(8, 64, 4, 128).transpose(1, 0, 2, 3).reshape(64, 4096)
    ).astype(np.float16)
    # bx2rb[df, b*16+m] = bx2[b*64+df]
    bx2rb = np.ascontiguousarray(
        np.repeat(bx2.reshape(8, 64).T[:, :, None], 16, axis=2).reshape(64, 128)
    ).astype(np.float16)
    wqa = np.empty((6, HQ), np.float16)
    bq2r = np.empty((3, HQ), np.float32)
    wq2 = np.empty((HQ, 3 * HQ), np.float32)
    qs = []
    for br, (qk, w1k, b1k, w2k, b2k) in enumerate([
            ("quad_x0", "Wq01", "bq01", "Wq02", "bq02"),
            ("quad_x1", "Wq11", "bq11", "Wq12", "bq12"),
            ("quad_x2", "Wq21", "bq21", "Wq22", "bq22")]):
        wqa[2 * br] = f(w1k)[0]
        wqa[2 * br + 1] = f(b1k)
        wq2[:, br * HQ:(br + 1) * HQ] = f(w2k)
        bq2r[br] = f(b2k)
        qs.append(f(qk)[:, 0])
    eqb = np.full((128, 1), eq, np.float32)

    shared = dict(wx1a=wx1a, wx2tb=wx2tb, bx2rb=bx2rb, wqa=wqa, wq2=wq2,
                  bq2r=bq2r, eqb=eqb)
    in_maps = []
    ones_row = np.ones((1, BL), np.float32)
    for c in range(NCORES):
        ish = inputx[c * BL:(c + 1) * BL]                        # [8192, 3]
        xTm = np.concatenate([ish.T, ones_row], axis=0)          # [4, 8192]
        qxa = np.empty((6, NL), np.float32)
        qxc = np.empty((128, 3 * NT), np.float32)
        for br in range(3):
            sh = qs[br][c * NL:(c + 1) * NL]
            qxa[2 * br] = sh
            qxa[2 * br + 1] = 1.0
            qxc[:, br * NT:(br + 1) * NT] = sh.reshape(NT, 128).T
        m = dict(shared)
        m["xT"] = np.ascontiguousarray(xTm).astype(np.float16)
        m["qxa"] = qxa.astype(np.float16)
        m["qxc"] = np.ascontiguousarray(qxc)
        in_maps.append(m)
    return in_maps


def _run(inputs, **kw):
    nc = _get_nc()
    in_maps = _prep_in_maps(inputs)
    res = run_bass_kernel_spmd(nc, in_maps, list(range(NCORES)), **kw)
    out = np.concatenate([res.results[c]["out"].reshape(-1)
                          for c in range(NCORES)]).astype(np.float32)
    return out, res


def kernel(**inputs) -> np.ndarray:
    out, _ = _run(inputs)
    return out


def kernel_traced(**inputs):
    """Correctness + NTFF profile (exec_time_ns) in one run."""
    return _run(inputs, trace=True)



# revision 7
# speedup vs baseline: 1.1925x; 1.1925x over previous
"""Trainium2 Bass kernel for nn_DecGreenNet_product_CP3 (collective-free).

Reference computation:
    lhs  = tanh(input @ Wx1 + bx1) @ Wx2 + bx2          # [B, 512]
    s_i  = sum_n sin(pi*eq*qx_n) * mlp_i(qx_n)           # [8,16] per branch
    rhs  = einsum('bx,dx,fx->bdf', s_a, s_c, s_e)        # [512]
    out  = lhs @ rhs                                     # [B]

Restructurings (validated to ~2e-3 rel err in numpy):
  1. out[b] = tanh(input[b] @ Wx1 + bx1) @ (Wx2 @ rhs) + bx2 @ rhs
     collapses the [B,512]x[512,512] GEMM into a matvec.
  2. The quad branch z[h] = sum_n sin(pi*eq*t_n) tanh(t_n W_h + b_h) is a
     1-D quadrature over 8192 nodes.  Nodes are host-SORTED (layout only)
     into 256 equal-count bins of 32; on device F(t) is expanded to 2nd
     order around each bin's 16th element c_m:
        z[h] = sum_m [32*G0(c_m,h) + G1'(c_m,h)*S1_m + G2'(c_m,h)*S2_m]
     with per-bin moments S1 = sum(t-c), S2 = sum (t-c)^2 (device DVE) and
     tanh needed only at 256*128 points (one small ACT).  Every core
     computes the full quadrature locally => NO collective (the NRT
     barrier + AllReduce cost ~80us on this platform).

Sharding: batch B split 8 ways (8192 rows/core); quad replicated.
"""

import os
import numpy as np

import concourse.bacc as bacc
import concourse.bass as bass
import concourse.mybir as mybir
import concourse.tile as tile
from concourse.bass_utils import run_bass_kernel_spmd

F32 = mybir.dt.float32
F16 = mybir.dt.float16
AF = mybir.ActivationFunctionType
ALU = mybir.AluOpType

NCORES = 8
B, DIN, H = 65536, 3, 512
N, HQ = 8192, 128
S0, RX = 8, 16
BL = B // NCORES          # 8192 batch rows per core
BINS, BW = 256, 32        # quad bins (2 halves of 128) x elems per bin
CH = 512                  # batch chunk (columns per matmul)
NCH = BL // CH            # 16 chunks
L1_EARLY = 4              # L1 chunks emitted before the quad z block

# fp16 scaling (same recipe as validated in numpy):
RC_SCALE = 2.0 ** -36     # rhs_vec -> r16 before the fp16 w-matmuls
OUT_SCALE = 2.0 ** 36     # undone in the final output pass
G1S = 32.0                # G1 tables /32, S1 *32
G2S = 1024.0              # G2 tables /1024, S2 *1024

# minimax odd polynomial for sin(t), t in [0, pi]: sin(t)=t*P(t^2), err<2e-5
SIN_C = (0.999984590176674, -0.16663258473611252, 8.312385898666645e-03,
         -1.9316230946716391e-04, 2.1732361127812407e-06)
# cos(t) = C(t^2), t in [0, pi], err < 3e-8
COS_C = (0.9999999738769563, -0.49999985121528656, 0.04166646225664891,
         -0.0013887731348778735, 2.4769044032458346e-05,
         -2.707535992044894e-07, 1.7243420044310883e-09)

_CACHED_NC = None
_STAGE = os.environ.get("K_STAGE", "full")  # tau | zs | w | full


def _ap(t, offset_cols, shape_ap):
    """Manual AP into tile t: shape_ap = [[pstride, np], [stride, n], ...]."""
    base = t[:, 0:1] if len(t.shape) == 2 else t
    return bass.AP(tensor=base.tensor, offset=base.offset + offset_cols,
                   ap=shape_ap)


def _build():
    nc = bacc.Bacc("TRN2", target_bir_lowering=False, debug=False,
                   num_devices=NCORES)

    xT = nc.dram_tensor("xT", [DIN + 1, BL], F16, kind="ExternalInput").ap()
    wx1a = nc.dram_tensor("wx1a", [DIN + 1, H], F16, kind="ExternalInput").ap()
    wx2t = nc.dram_tensor("wx2tb", [64, 4096], F16, kind="ExternalInput").ap()
    bx2r = nc.dram_tensor("bx2rb", [64, 128], F16, kind="ExternalInput").ap()
    qts = nc.dram_tensor("qts", [128, 192], F32, kind="ExternalInput").ap()
    cm1 = nc.dram_tensor("cm1", [2, 768], F32, kind="ExternalInput").ap()
    wb2 = nc.dram_tensor("wb2", [2, 384], F32, kind="ExternalInput").ap()
    wq1r = nc.dram_tensor("wq1r", [128, 384], F32, kind="ExternalInput").ap()
    wq2 = nc.dram_tensor("wq2", [HQ, 3 * HQ], F32, kind="ExternalInput").ap()
    bq2r = nc.dram_tensor("bq2r", [3, HQ], F32, kind="ExternalInput").ap()
    eqb = nc.dram_tensor("eqb", [128, 1], F32, kind="ExternalInput").ap()
    out_d = nc.dram_tensor("out", [BL], F32, kind="ExternalOutput").ap()

    global _APS
    _APS = (xT, wx1a, wx2t, bx2r, qts, cm1, wb2, wq1r, wq2, bq2r, eqb, out_d)
    with tile.TileContext(nc) as tc:
        _body(nc, tc)
    nc.compile()
    return nc


def _body(nc, tc):
    xT, wx1a, wx2t, bx2r, qts, cm1, wb2, wq1r, wq2, bq2r, eqb, out_d = _APS
    with (
        tc.tile_pool(name="const", bufs=1) as constp,
        tc.tile_pool(name="qsb", bufs=1) as qsb,
        tc.tile_pool(name="gsb", bufs=1) as gsb,
        tc.tile_pool(name="dram", bufs=1, space="DRAM") as dram,
        tc.tile_pool(name="tinyp", bufs=1, space="PSUM") as tinyp,
        tc.tile_pool(name="mainsb", bufs=1) as mainsb,
        tc.tile_pool(name="orowp", bufs=3) as orowp,
        tc.tile_pool(name="esb", bufs=2) as esb,
        tc.tile_pool(name="hidp", bufs=32) as hidp,
        tc.tile_pool(name="prep", bufs=2, space="PSUM") as prep,
        tc.tile_pool(name="outp", bufs=2, space="PSUM") as outpp,
    ):
        ones128 = constp.tile([128, 1], F32)
        nc.vector.memset(ones128, 1.0)
        ones16 = constp.tile([128, 1], F16)
        nc.vector.memset(ones16, 1.0)
        warm = constp.tile([128, 1], F32)
        nc.scalar.activation(out=warm, in_=ones128, func=AF.Tanh)

        # ---------------- input DMAs ----------------
        qts_sb = qsb.tile([128, 192], F32, tag="qts")
        nc.sync.dma_start(out=qts_sb, in_=qts)
        cm1_sb = qsb.tile([2, 768], F32, tag="cm1")
        nc.sync.dma_start(out=cm1_sb, in_=cm1)
        wb2_sb = qsb.tile([2, 384], F32, tag="wb2")
        nc.sync.dma_start(out=wb2_sb, in_=wb2)
        eqb_sb = qsb.tile([128, 1], F32, tag="eqb")
        nc.sync.dma_start(out=eqb_sb, in_=eqb)
        bq2r_sb = []
        for br in range(3):
            t = qsb.tile([1, HQ], F32, tag=f"bq2r{br}")
            nc.sync.dma_start(out=t, in_=bq2r[br:br + 1, :])
            bq2r_sb.append(t)
        wq2_sb = qsb.tile([HQ, 3 * HQ], F32, tag="wq2")
        nc.sync.dma_start(out=wq2_sb, in_=wq2)

        xT_sb = mainsb.tile([DIN + 1, BL], F16, tag="xT")
        nc.gpsimd.dma_start(out=xT_sb, in_=xT)
        wx1a_sb = mainsb.tile([DIN + 1, H], F16, tag="wx1a")
        nc.gpsimd.dma_start(out=wx1a_sb, in_=wx1a)
        wq1r_sb = qsb.tile([128, 384], F32, tag="wq1r")
        nc.gpsimd.dma_start(out=wq1r_sb, in_=wq1r)
        wx2t_sb = mainsb.tile([64, 4096], F16, tag="wx2t")
        nc.gpsimd.dma_start(out=wx2t_sb, in_=wx2t)
        bx2r_sb = mainsb.tile([64, 128], F16, tag="bx2r")
        nc.gpsimd.dma_start(out=bx2r_sb, in_=bx2r)

        def qdump(ap2d):
            p, c = ap2d.shape[0], ap2d.shape[1]
            nc.sync.dma_start(
                out=out_d[0:p * c].rearrange("(p c) -> p c", c=c),
                in_=ap2d)

        # ---------------- u matmuls + tau ----------------
        uq = prep.tile([128, 1024], F32, tag="pre")
        for blk in range(6):
            br = blk // 2
            nc.tensor.matmul(
                uq[:, blk * 128:(blk + 1) * 128],
                lhsT=cm1_sb[:, blk * 128:(blk + 1) * 128],
                rhs=wb2_sb[:, br * 128:(br + 1) * 128],
                start=True, stop=True)
        tau = gsb.tile([128, 768], F32, tag="tau")
        nc.scalar.activation(out=tau, in_=uq[:, 0:768], func=AF.Tanh)
        if _STAGE == "tau":
            qdump(tau[:, 0:8])
            return

        # ---------------- scalar columns (DVE) ----------------
        # eq-dependent per-partition scalars
        eqpi = qsb.tile([128, 1], F32, tag="eqpi")
        nc.vector.tensor_scalar_mul(eqpi, eqb_sb, float(np.pi))
        eqpi2 = qsb.tile([128, 1], F32, tag="eqpi2")
        nc.vector.tensor_tensor(out=eqpi2, in0=eqpi, in1=eqpi, op=ALU.mult)
        # ypph scalar = -(pi eq)^2/(2*G2S) / 32 applied to y32 columns
        eqpi2m = qsb.tile([128, 1], F32, tag="eqpi2m")
        nc.vector.tensor_scalar_mul(eqpi2m, eqpi2, float(-1.0 / (2 * G2S * 32.0)))
        eqpi_d32 = qsb.tile([128, 1], F32, tag="eqpid32")
        nc.vector.tensor_scalar_mul(eqpi_d32, eqpi, float(1.0 / G1S))
        eqpi_d1k = qsb.tile([128, 1], F32, tag="eqpid1k")
        nc.vector.tensor_scalar_mul(eqpi_d1k, eqpi, float(1.0 / G2S))

        # t*pi*eq over all sorted nodes [128, 192]; sin -> y_all
        targ = qsb.tile([128, 192], F32, tag="targ")
        nc.vector.tensor_scalar_mul(targ, qts_sb, eqpi[:, 0:1])
        uu = qsb.tile([128, 192], F32, tag="uu")
        nc.vector.tensor_tensor(out=uu, in0=targ, in1=targ, op=ALU.mult)
        c1, c3, c5, c7, c9 = [float(v) for v in SIN_C]
        pp = qsb.tile([128, 192], F32, tag="pp")
        nc.vector.tensor_scalar_mul(pp, uu, c9)
        for cof in (c7, c5, c3):
            nc.vector.scalar_tensor_tensor(
                out=pp, in0=pp, scalar=cof, in1=uu, op0=ALU.add, op1=ALU.mult)
        y_all = qsb.tile([128, 192], F32, tag="yall")
        nc.vector.scalar_tensor_tensor(
            out=y_all, in0=pp, scalar=c1, in1=targ, op0=ALU.add, op1=ALU.mult)
        ysum = qsb.tile([128, 3], F32, tag="ysum")
        for br in range(3):
            nc.vector.tensor_reduce(
                out=ysum[:, br:br + 1], in_=y_all[:, br * 64:(br + 1) * 64],
                axis=mybir.AxisListType.X, op=ALU.add)

        # strided views of the 6 c-columns (16th elem of each bin block)
        def c6_of(t):
            return _ap(t, 16, [t[:, 0:1].ap[0], [32, 6]])
        targ6 = c6_of(targ)
        u6 = c6_of(uu)
        p4_6 = c6_of(pp)

        # y-variant columns [128, 6]: (p4+c1) * (k * targ)
        yv = qsb.tile([128, 24], F32, tag="yv")  # y32 | y/32 | -y/1024 | ypph
        tscaled = qsb.tile([128, 18], F32, tag="tsc")
        for i, k in enumerate((32.0, 1.0 / G1S, -1.0 / G2S)):
            nc.vector.tensor_scalar_mul(tscaled[:, i * 6:(i + 1) * 6], targ6, k)
            nc.vector.scalar_tensor_tensor(
                out=yv[:, i * 6:(i + 1) * 6], in0=p4_6, scalar=c1,
                in1=tscaled[:, i * 6:(i + 1) * 6], op0=ALU.add, op1=ALU.mult)
        # ypph = y32 * (-(pi eq)^2/(2*G2S*32))  (per-partition scalar)
        nc.vector.tensor_scalar_mul(yv[:, 18:24], yv[:, 0:6], eqpi2m[:, 0:1])

        # cos chain on u6 -> yp variants [128, 6]
        cq = qsb.tile([128, 6], F32, tag="cq")
        k0 = float(COS_C[6])
        nc.vector.tensor_scalar_mul(cq, u6, k0)
        for cof in COS_C[5:0:-1]:
            nc.vector.scalar_tensor_tensor(
                out=cq, in0=cq, scalar=float(cof), in1=u6,
                op0=ALU.add, op1=ALU.mult)
        ypv = qsb.tile([128, 12], F32, tag="ypv")  # yp/32 | yp/1024
        for i, sc in enumerate((eqpi_d32, eqpi_d1k)):
            nc.vector.scalar_tensor_tensor(
                out=ypv[:, i * 6:(i + 1) * 6], in0=cq, scalar=float(COS_C[0]),
                in1=_ap(sc, 0, [sc.ap[0], [0, 6]]), op0=ALU.add, op1=ALU.mult)

        # W^2 replicated (device)
        w2sq = qsb.tile([128, 384], F32, tag="w2sq")
        nc.vector.tensor_tensor(out=w2sq, in0=wq1r_sb, in1=wq1r_sb, op=ALU.mult)

        # ---------------- S moments (GpSimd) ----------------
        S_sb = qsb.tile([128, 12], F16, tag="Ssb")  # (br,q) -> S1,S2 cols
        dtmp = qsb.tile([128, 64], F32, tag="dtmp")
        for blk in range(6):
            t_sl = qts_sb[:, blk * 32:(blk + 1) * 32]
            c_ap = qts_sb[:, blk * 32 + 16:blk * 32 + 17]
            d32 = dtmp[:, 0:32] if blk % 2 == 0 else dtmp[:, 32:64]
            nc.gpsimd.tensor_scalar(
                out=d32, in0=t_sl, scalar1=c_ap, scalar2=float(G1S),
                op0=ALU.subtract, op1=ALU.mult)
            with nc.allow_low_precision(reason="fp32-internal reduce to fp16"):
                nc.vector.tensor_reduce(
                    out=S_sb[:, blk * 2:blk * 2 + 1], in_=d32,
                    axis=mybir.AxisListType.X, op=ALU.add)
            d2 = dtmp[:, 0:32] if blk % 2 == 1 else dtmp[:, 32:64]
            nc.gpsimd.tensor_tensor(out=d2, in0=d32, in1=d32, op=ALU.mult)
            with nc.allow_low_precision(reason="fp32-internal reduce to fp16"):
                nc.vector.tensor_reduce(
                    out=S_sb[:, blk * 2 + 1:blk * 2 + 2], in_=d2,
                    axis=mybir.AxisListType.X, op=ALU.add)

        # ---------------- G tables (DVE) ----------------
        G0 = gsb.tile([128, 768], F16, tag="G0")
        G1 = gsb.tile([128, 768], F16, tag="G1")
        G2 = gsb.tile([128, 768], F16, tag="G2")
        tA = gsb.tile([128, 128], F32, tag="tA")
        tB = gsb.tile([128, 128], F32, tag="tB")
        tC = gsb.tile([128, 128], F32, tag="tC")
        for blk in range(6):
            br, sl = blk // 2, slice(blk * 128, (blk + 1) * 128)
            wsl = slice(br * 128, (br + 1) * 128)
            tq = tau[:, sl]
            y32 = yv[:, 0 + blk:1 + blk]
            yd32 = yv[:, 6 + blk:7 + blk]
            ym1k = yv[:, 12 + blk:13 + blk]
            ypph = yv[:, 18 + blk:19 + blk]
            ypd32 = ypv[:, 0 + blk:1 + blk]
            ypd1k = ypv[:, 6 + blk:7 + blk]
            # tA = tau^2 ; tA = 1-tau^2 (P)
            nc.vector.tensor_tensor(out=tA, in0=tq, in1=tq, op=ALU.mult)
            nc.vector.tensor_scalar(out=tA, in0=tA, scalar1=-1.0, scalar2=1.0,
                                    op0=ALU.mult, op1=ALU.add)
            # G0 = y32 * tau
            nc.vector.tensor_scalar_mul(G0[:, sl], tq, y32)
            # tB = W*P ; tC = (y/32)*tB ; G1 = (yp/32)*tau + tC
            nc.vector.tensor_tensor(out=tB, in0=wq1r_sb[:, wsl], in1=tA,
                                    op=ALU.mult)
            nc.vector.tensor_scalar_mul(tC, tB, yd32)
            nc.vector.scalar_tensor_tensor(
                out=G1[:, sl], in0=tq, scalar=ypd32, in1=tC,
                op0=ALU.mult, op1=ALU.add)
            # tA = tau*P ; tA = W^2*(tau*P) ; tC = ypph*tau ;
            # tC = (yp/1024)*tB + tC ; G2 = ym1k? no: G2 = (-y/1024)*tA + tC
            nc.vector.tensor_tensor(out=tA, in0=tq, in1=tA, op=ALU.mult)
            nc.vector.tensor_tensor(out=tA, in0=w2sq[:, wsl], in1=tA,
                                    op=ALU.mult)
            nc.vector.tensor_scalar_mul(tC, tq, ypph)
            nc.vector.scalar_tensor_tensor(
                out=tC, in0=tB, scalar=ypd1k, in1=tC, op0=ALU.mult, op1=ALU.add)
            nc.vector.scalar_tensor_tensor(
                out=G2[:, sl], in0=tA, scalar=ym1k, in1=tC,
                op0=ALU.mult, op1=ALU.add)

        # ---------------- main L1 chunks (first batch) ----------------
        hid_tiles = {}

        def emit_l1(c):
            tiles = []
            for half in range(2):
                pre = prep.tile([128, 1024], F32, tag="pre")
                for k in range(2):
                    ht = half * 2 + k
                    nc.tensor.matmul(
                        pre[:, k * 512:(k + 1) * 512],
                        lhsT=wx1a_sb[:, ht * 128:(ht + 1) * 128],
                        rhs=xT_sb[:, c * CH:(c + 1) * CH],
                        start=True, stop=True)
                hid = hidp.tile([128, 1024], F16, tag="hid")
                nc.scalar.activation(out=hid, in_=pre, func=AF.Tanh)
                tiles.append(hid)
            hid_tiles[c] = tiles

        for c in range(L1_EARLY):
            emit_l1(c)

        # ---------------- z, sy, s (PE + copies) ----------------
        qsmall = tinyp.tile([128, 12], F32, tag="tiny")
        for br in range(3):
            for q in range(2):
                blk = br * 2 + q
                sl = slice(blk * 128, (blk + 1) * 128)
                for k, rhs_col in (
                        (0, ones16[:, 0:1]),
                        (1, S_sb[:, blk * 2:blk * 2 + 1]),
                        (2, S_sb[:, blk * 2 + 1:blk * 2 + 2])):
                    nc.tensor.matmul(
                        qsmall[:, br:br + 1],
                        lhsT=(G0, G1, G2)[k][:, sl],
                        rhs=rhs_col,
                        start=(q == 0 and k == 0), stop=(q == 1 and k == 2))
            # sy: partition-reduce of ysum via ones matmul
            nc.tensor.matmul(
                qsmall[0:1, 3 + br:4 + br], lhsT=ysum[:, br:br + 1],
                rhs=ones128[:, 0:1], start=True, stop=True)
        z_sb = qsb.tile([128, 6], F32, tag="zsb")
        nc.vector.tensor_copy(out=z_sb, in_=qsmall[:, 0:6])
        if _STAGE == "zs":
            qdump(z_sb[:, 0:6])
            return
        for br in range(3):
            nc.tensor.matmul(
                qsmall[:, 6 + br:7 + br],
                lhsT=wq2_sb[:, br * HQ:(br + 1) * HQ],
                rhs=z_sb[:, br:br + 1], start=True, stop=False)
            nc.tensor.matmul(
                qsmall[:, 6 + br:7 + br],
                lhsT=bq2r_sb[br],
                rhs=z_sb[0:1, 3 + br:4 + br], start=False, stop=True)
        s_sb = qsb.tile([128, 3], F32, tag="ssb")
        nc.vector.tensor_copy(out=s_sb, in_=qsmall[:, 6:9])

        # ---------------- s transpose via DRAM bounce ----------------
        sb_d = dram.tile([16, 24], F32, tag="sbounce")
        nc.gpsimd.dma_start(out=sb_d.rearrange("x (c b) -> b x c", b=8),
                            in_=s_sb)
        sT_sb = esb.tile([16, 24], F32, tag="sT")
        nc.gpsimd.dma_start(out=sT_sb, in_=sb_d)

        # ---------------- einsum -> rhs_vec -> w, c ----------------
        sc_ap = sT_sb[:, 8:16]
        se_ap = sT_sb[:, 16:24]
        in0 = bass.AP(tensor=sc_ap.tensor, offset=sc_ap.offset,
                      ap=[sc_ap.ap[0], sc_ap.ap[1], [0, 8]])
        in1 = bass.AP(tensor=se_ap.tensor, offset=se_ap.offset,
                      ap=[se_ap.ap[0], [0, 8], se_ap.ap[1]])
        E_sb = esb.tile([16, 64], F32, tag="E")
        nc.vector.tensor_tensor(
            out=E_sb.rearrange("p (d f) -> p d f", f=8),
            in0=in0, in1=in1, op=ALU.mult)
        rhsp = tinyp.tile([64, 8], F32, tag="tiny")
        nc.tensor.matmul(rhsp, lhsT=E_sb, rhs=sT_sb[:, 0:8],
                         start=True, stop=True)
        r16 = esb.tile([64, 8], F16, tag="r16")
        nc.vector.tensor_scalar_mul(r16, rhsp, float(RC_SCALE))
        wps = tinyp.tile([128, 4], F32, tag="tiny")
        for it in range(4):
            for b in range(8):
                nc.tensor.matmul(
                    wps[:, it:it + 1],
                    lhsT=wx2t_sb[:, b * 512 + it * 128:b * 512 + (it + 1) * 128],
                    rhs=r16[:, b:b + 1],
                    start=(b == 0), stop=(b == 7))
        w_sb = esb.tile([128, 4], F16, tag="wsb")
        nc.vector.tensor_copy(out=w_sb, in_=wps)
        c16p = tinyp.tile([16, 1], F32, tag="tiny")
        for b in range(8):
            nc.tensor.matmul(
                c16p, lhsT=bx2r_sb[:, b * 16:(b + 1) * 16],
                rhs=r16[:, b:b + 1],
                start=(b == 0), stop=(b == 7))
        c16_sb = esb.tile([16, 1], F32, tag="c16")
        nc.vector.tensor_copy(out=c16_sb, in_=c16p)
        if _STAGE == "w":
            qdump(w_sb.bitcast(F32)[:, 0:2])
            return

        # ---------------- rest of L1 + dots ----------------
        def emit_dot(c):
            op = outpp.tile([1, 512], F32, tag="outp")
            for ht in range(4):
                nc.tensor.matmul(
                    op,
                    lhsT=w_sb[:, ht:ht + 1],
                    rhs=hid_tiles[c][ht // 2][:, (ht % 2) * 512:(ht % 2 + 1) * 512],
                    start=(ht == 0), stop=(ht == 3))
            orow = orowp.tile([1, 512], F32, tag="outrow")
            nc.vector.tensor_scalar(
                out=orow, in0=op, scalar1=c16_sb[0:1, 0:1],
                scalar2=float(OUT_SCALE), op0=ALU.add, op1=ALU.mult)
            nc.sync.dma_start(
                out=out_d[c * CH:(c + 1) * CH].rearrange("(o b) -> o b", o=1),
                in_=orow)

        for c in range(L1_EARLY, NCH):
            emit_l1(c)
        for c in range(NCH):
            emit_dot(c)


def _get_nc():
    global _CACHED_NC
    if _CACHED_NC is None:
        _CACHED_NC = _build()
    return _CACHED_NC


def _prep_in_maps(inputs):
    f = lambda k: np.ascontiguousarray(np.asarray(inputs[k], np.float32))
    inputx = f("input")
    eq = float(np.asarray(inputs["eq_param"]).reshape(-1)[0])
    Wx1, bx1 = f("Wx1"), f("bx1")
    Wx2, bx2 = f("Wx2"), f("bx2")

    wx1a = np.concatenate([Wx1, bx1[None, :]], axis=0).astype(np.float16)
    # wx2tb[df, b*512+it*128+i] = Wx2T[b*64+df, it*128+i]
    wx2tb = np.ascontiguousarray(
        Wx2.T.reshape(8, 64, 4, 128).transpose(1, 0, 2, 3).reshape(64, 4096)
    ).astype(np.float16)
    # bx2rb[df, b*16+m] = bx2[b*64+df]
    bx2rb = np.ascontiguousarray(
        np.repeat(bx2.reshape(8, 64).T[:, :, None], 16, axis=2).reshape(64, 128)
    ).astype(np.float16)

    qts = np.empty((128, 192), np.float32)
    cm1 = np.empty((2, 768), np.float32)
    cm1[1] = 1.0
    wb2 = np.empty((2, 384), np.float32)
    wq1r = np.empty((128, 384), np.float32)
    wq2 = np.empty((HQ, 3 * HQ), np.float32)
    bq2r = np.empty((3, HQ), np.float32)
    for br, (qk, w1k, b1k, w2k, b2k) in enumerate([
            ("quad_x0", "Wq01", "bq01", "Wq02", "bq02"),
            ("quad_x1", "Wq11", "bq11", "Wq12", "bq12"),
            ("quad_x2", "Wq21", "bq21", "Wq22", "bq22")]):
        srt = np.sort(f(qk)[:, 0])
        arr = srt.reshape(2, 128, BW)          # [half, bin-in-half, elem]
        for q in range(2):
            blk = br * 2 + q
            qts[:, blk * 32:(blk + 1) * 32] = arr[q]
            cm1[0, blk * 128:(blk + 1) * 128] = arr[q, :, BW // 2]
        wb2[0, br * 128:(br + 1) * 128] = f(w1k)[0]
        wb2[1, br * 128:(br + 1) * 128] = f(b1k)
        wq1r[:, br * 128:(br + 1) * 128] = np.broadcast_to(
            f(w1k)[0][None, :], (128, 128))
        wq2[:, br * HQ:(br + 1) * HQ] = f(w2k)
        bq2r[br] = f(b2k)
    eqb = np.full((128, 1), eq, np.float32)

    shared = dict(wx1a=wx1a, wx2tb=wx2tb, bx2rb=bx2rb, qts=qts, cm1=cm1,
                  wb2=wb2, wq1r=wq1r, wq2=wq2, bq2r=bq2r, eqb=eqb)
    in_maps = []
    ones_row = np.ones((1, BL), np.float32)
    for c in range(NCORES):
        ish = inputx[c * BL:(c + 1) * BL]                        # [8192, 3]
        xTm = np.concatenate([ish.T, ones_row], axis=0)          # [4, 8192]
        m = dict(shared)
        m["xT"] = np.ascontiguousarray(xTm).astype(np.float16)
        in_maps.append(m)
    return in_maps


def _run(inputs, **kw):
    nc = _get_nc()
    in_maps = _prep_in_maps(inputs)
    res = run_bass_kernel_spmd(nc, in_maps, list(range(NCORES)), **kw)
    out = np.concatenate([res.results[c]["out"].reshape(-1)
                          for c in range(NCORES)]).astype(np.float32)
    return out, res


def kernel(**inputs) -> np.ndarray:
    out, _ = _run(inputs)
    return out


def kernel_traced(**inputs):
    """Correctness + NTFF profile (exec_time_ns) in one run."""
    return _run(inputs, trace=True)
